# revision 7
# baseline (speedup 1.0000x reference)
"""Causal multi-head attention + RoPE — Trainium2 Bass kernel, 8-core SPMD.

Sharding: batch (2) x head-groups (4 heads each) -> 8 cores.
Wq/Wk/Wv are column-sharded per head group, Wo row-sharded; each core
computes a partial out-projection [S, D] and the host sums the 4
partials per batch (the "all-reduce after out_proj").

Per-core device pipeline (projections bf16, scores fp8 DoubleRow):
  1. x^T lands via combined strided DMAs, first-512-seq-columns first,
     so Q+K projections for the first q-range run progressively off a
     ~3us transfer and the first Exp issues at ~8us. Weight rows are
     host-permuted head-major so even/odd feature pairs share a
     partition (qf0[32h+i] = q_{h,2i}, qf1[32h+i] = q_{h,2i+1}).
  2. RoPE (q-side DVE, k-side GpSimd) writes 16-scaled fp8 rotated
     halves straight into qcd/kcd [128, 2S] (re block | ro block).
     Scores = ONE fp8 DoubleRow matmul per head per k-block: the
     contraction pairs (even_i, odd_i) sit on one partition with the
     pair split along the free axis, so no head-regroup matmuls exist.
     Causal mask = a second fp8 DoubleRow matmul accumulating -3.67e6
     onto the diagonal block's upper triangle (Exp underflows to 0).
  3. ScalarE runs ONLY Exp (scores bounded => max-pass free), one
     merged instruction per k-block covering both heads. Exp'd tiles
     flow through a GLOBAL 16-deep cross-q-range prefetch queue (exp
     work is backloaded: the last q-range holds 40% of it). Remaining
     projection work is chopped into <=900ns chains metered one at a
     time between score emissions — the 2-buf score-tile rotation
     gives ScalarE only ~1.9us of runway, so any larger PE bunch
     would starve it.
  4. AV transposed in bf16: stationary = exp'd scores, moving =
     [V_h | 1]: O[q,f]+denominator accumulate in a 2-bank tile; one
     strided-AP reciprocal per PSUM bank (4 denominators at once),
     per-partition scalar-multiply normalizes, a PE transpose restores
     O^T feature-major. Bank-0 divides start two k-blocks before the
     pair ends; bank-1 + out-projection ride into the next pair.
  5. partial = O_norm @ Wo^T per 2-s-block slab with per-s-block bf16
     DMAs out; the host sums the 4 partials per batch.
"""

import os
import sys
from contextlib import ExitStack

import numpy as np

for _p in ("/opt/trn_rl_repo", "/root/.axon_site/_ro/trn_rl_repo"):
    if os.path.isdir(_p) and _p not in sys.path:
        sys.path.insert(0, _p)

import ml_dtypes  # noqa: E402
import concourse.bass as bass  # noqa: E402
import concourse.tile as tile  # noqa: E402
from concourse import bacc, mybir  # noqa: E402
from concourse.bass_utils import run_bass_kernel_spmd  # noqa: E402

BF16 = mybir.dt.bfloat16
F32 = mybir.dt.float32
F8E4 = mybir.dt.float8e4
F8E5 = mybir.dt.float8e5
AF = mybir.ActivationFunctionType
DR = mybir.MatmulPerfMode.DoubleRow

B, S, D = 2, 2048, 1024
H, DK = 16, 64
HPC = 4                # heads per core
HF = HPC * DK          # 256 projected features per core
N_CORES = 8
THETA = 10000.0
SCALE = 1.0 / (DK ** 0.5)
QKS = 16.0             # fp8 q/k prescale (folded into cos/sin tables)
EXPS = SCALE / (QKS * QKS)

KD = D // 128          # 8 contraction chunks for projections
NS = S // 512          # 4 q-ranges of 512
SB = S // 128          # 16 s-blocks of 128
PF = 22                # global exp'd-score prefetch lookahead (SBUF tiles)


# ---------------------------------------------------------------------------
# Device program (identical on all 8 cores; only the input shards differ)
# ---------------------------------------------------------------------------
def _build_program():
    nc = bacc.Bacc("TRN2", target_bir_lowering=False, debug=False,
                   num_devices=N_CORES)

    x8A = nc.dram_tensor("x8A", [2 * D, S], F8E4, kind="ExternalInput")
    w8A = nc.dram_tensor("w8A", [128, 6 * KD * HF], F8E4,
                         kind="ExternalInput")
    woA = nc.dram_tensor("woA", [128, 2 * D], BF16, kind="ExternalInput")
    # [cos0|sin0|ident | cos1|sin1 | cos2|sin2 | cos3|sin3] (x0.25)
    csA = nc.dram_tensor("csA", [128, 2 * S + 128], BF16,
                         kind="ExternalInput")
    # [ident_dr(2x128) | trineg_dr(2x128, e5m2 bytes)]
    c8A = nc.dram_tensor("c8A", [128, 512], F8E4, kind="ExternalInput")
    out = nc.dram_tensor("out", [S, D], BF16, kind="ExternalOutput")

    with tile.TileContext(nc) as tc, ExitStack() as ctx:
        cons = ctx.enter_context(tc.tile_pool(name="cons", bufs=1))

        # ---- persistent SBUF tensors -----------------------------------
        # x^T (fp8 value + fp8 residual) as chunk-major tiles so strided
        # DMAs can land the first 512 seq-cols of ALL 8 d-chunks at once
        xa8 = cons.tile([128, 2 * KD * S], F8E4, tag="xa8", name="xa8")
        xah, xal = xa8[:, 0:KD * S], xa8[:, KD * S:2 * KD * S]
        wall = cons.tile([128, 6 * KD * HF], F8E4, tag="wall", name="wall")
        WOFF = {"wqh": 0, "wkh": 2048, "wql": 4096, "wkl": 6144,
                "wvh": 8192, "wvl": 10240}
        wsb = {k: wall[:, WOFF[k]:WOFF[k] + KD * HF] for k in WOFF}
        woa = cons.tile([128, 2 * D], BF16, tag="woa", name="woa")
        csa = cons.tile([128, 2 * S + 128], BF16, tag="csa", name="csa")

        def cs_base(nch):
            return 1152 + 1024 * (nch - 1) if nch else 0

        def cos_t(nch):
            b = cs_base(nch)
            return csa[:, b:b + 512]

        def sin_t(nch):
            b = cs_base(nch) + 512
            return csa[:, b:b + 512]

        ident = csa[:, 1024:1152]
        c8a = cons.tile([128, 512], F8E4, tag="c8a", name="c8a")
        ident_dr = c8a[0:64, 0:256].rearrange("p (t n) -> p t n", t=2)
        trineg_dr = c8a.bitcast(F8E5)[0:64, 256:512].rearrange(
            "p (t n) -> p t n", t=2)
        # bf16 pre-RoPE staging (even/odd halves, head-major partitions)
        qf = [cons.tile([128, S], BF16, tag=f"qf{m}", name=f"qf{m}")
              for m in range(2)]
        kf = [cons.tile([128, S], BF16, tag=f"kf{m}", name=f"kf{m}")
              for m in range(2)]
        # fp8 rotated q/k: [re block | ro block], 16x prescaled
        qcd = cons.tile([128, 2 * S], F8E4, tag="qcd", name="qcd")
        kcd = cons.tile([128, 2 * S], F8E4, tag="kcd", name="kcd")
        # V in natural layout, 65 columns per head (denom col appended)
        v_sb = [cons.tile([128, HPC * (DK + 1)], BF16, tag=f"v{s}",
                          name=f"v{s}")
                for s in range(SB)]
        # normalized O^T (features x S), two 128-feature tiles
        otb = [cons.tile([128, S], BF16, tag=f"otb{i}", name=f"otb{i}")
               for i in range(2)]

        # ---- input DMAs ------------------------------------------------
        # consolidated + strictly need-ordered; x value/residual chunks
        # land per seq-range via one 3D-strided transfer each
        def x_dma_p(xpart, c0, c1):
            # one 3D-strided transfer for the 8 d-chunks of one seq-range
            # of the fp8 value (xpart=0) or residual (xpart=1) plane
            base = KD * S * xpart
            d = xa8[:, base:base + KD * S].rearrange(
                "p (k s) -> p k s", k=KD)[:, :, c0:c1]
            sr = x8A[D * xpart:D * (xpart + 1), c0:c1].rearrange(
                "(k p) s -> p k s", p=128)
            nc.sync.dma_start(d, sr)

        def x_dma(c0, c1):
            x_dma_p(0, c0, c1)
            x_dma_p(1, c0, c1)

        # first x range: value plane on SP, residual on DVE (parallel)

        w2 = wall[:, 0:4096].rearrange("p (u c) -> p u c", u=2)
        wA2 = w8A[:, 0:4096].rearrange("p (u c) -> p u c", u=2)
        # first d-chunk of wq8h+wk8h, then their rests, then wq8l+wk8l
        # the SP sequencer can only issue one DMA per ~650ns, so the
        # startup-critical transfers fan out across three issue queues
        # (ScalarE idles until the first Exp, DVE until the first copy)
        nc.scalar.dma_start(w2[:, :, 0:256], wA2[:, :, 0:256])
        x_dma_p(0, 0, 512)
        d0 = xa8[:, KD * S:2 * KD * S].rearrange(
            "p (k s) -> p k s", k=KD)[:, :, 0:512]
        s0 = x8A[D:2 * D, 0:512].rearrange("(k p) s -> p k s", p=128)
        nc.scalar.dma_start(d0, s0)                                       # SP + DVE
        nc.scalar.dma_start(w2[:, :, 256:2048], wA2[:, :, 256:2048])
        nc.sync.dma_start(wall[:, 4096:8192], w8A[:, 4096:8192])
        nc.scalar.dma_start(csa[:, 0:1152], csA[:, 0:1152])  # cs0+ident
        nc.scalar.dma_start(c8a[:], c8A[:, :])   # mask before first diag
        nc.sync.dma_start(wall[:, 8192:12288], w8A[:, 8192:12288])  # wv
        x_dma(512, 1024)
        nc.sync.dma_start(csa[:, 1152:2176], csA[:, 1152:2176])  # cs1
        nc.sync.dma_start(woa[:], woA[:, :])   # out-proj qr0
        x_dma(1024, 1536)
        nc.sync.dma_start(csa[:, 2176:3200], csA[:, 2176:3200])  # cs2
        x_dma(1536, 2048)
        nc.sync.dma_start(csa[:, 3200:4224], csA[:, 3200:4224])  # cs3

        psum = ctx.enter_context(tc.tile_pool(name="psum", bufs=2,
                                              space="PSUM"))
        atp = ctx.enter_context(tc.tile_pool(name="atp", bufs=44))
        dvp = ctx.enter_context(tc.tile_pool(name="dvp", bufs=6))
        rp = ctx.enter_context(tc.tile_pool(name="rope", bufs=4))
        rpg = ctx.enter_context(tc.tile_pool(name="ropeg", bufs=4))
        osb = ctx.enter_context(tc.tile_pool(name="osb", bufs=2))

        # denominator columns: memset all up front (Pool idles during
        # the input DMA anyway; v_group then only writes the V columns)
        for s_ in range(SB):
            nc.gpsimd.memset(v_sb[s_][:], 1.0)

        def sc_tile(name):
            return psum.tile([128, 1024], F32, tag="sc", name=name, bufs=2)

        def fp_tile_bf(name):
            # bf16 view of an "fp"-class slot (transpose staging) — keeps
            # the divide-path transposes OUT of the score-tile rotation,
            # which the Exp stream is latency-coupled to
            return psum.tile([128, 1024], BF16, tag="fp", name=name, bufs=2)

        def oq_tile(name):
            return psum.tile([128, 1024], F32, tag="oq", name=name, bufs=1)

        def fp_tile(name):
            return psum.tile([128, 512], F32, tag="fp", name=name, bufs=2)

        # ---- building blocks -------------------------------------------
        # DoubleRow access patterns: contraction chunk-pairs (2j, 2j+1)
        # sit at free-axis stride S (x) / HF (weights)
        def w_ap(w, j, m):
            return wsb[w][:, 512 * j:512 * (j + 1)].rearrange(
                "p (t c) -> p t c", t=2)[:, :, 128 * m:128 * (m + 1)]

        def wv_ap(w, j):
            return wsb[w][:, 512 * j:512 * (j + 1)].rearrange(
                "p (t c) -> p t c", t=2)

        def x_ap(xa, j, c0, c1):
            return xa[:, 2 * S * j:2 * S * (j + 1)].rearrange(
                "p (t n) -> p t n", t=2)[:, :, c0:c1]

        # psum = x8h@W8h + x8h@W8l + x8l@W8h  (fp8 error feedback,
        # ~bf16 accuracy at half the PE rows per product)
        PRODS = (("h", "h"), ("h", "l"), ("l", "h"))

        def qk_group(dst, wn, m, nch):
            c0 = 512 * nch
            ps = fp_tile("qkg")
            for pi, (xp, wp) in enumerate(PRODS):
                xa = xah if xp == "h" else xal
                for j in range(KD // 2):
                    nc.tensor.matmul(
                        ps[:], w_ap(wn + wp, j, m),
                        x_ap(xa, j, c0, c0 + 512),
                        start=(pi == 0 and j == 0),
                        stop=(pi == 2 and j == KD // 2 - 1),
                        perf_mode=DR)
            nc.vector.tensor_copy(dst[m][:, c0:c0 + 512], ps[:])

        def v_group(s):
            ps = fp_tile("vg")
            for pi, (xp, wp) in enumerate(PRODS):
                xa = xah if xp == "h" else xal
                for j in range(KD // 2):
                    nc.tensor.matmul(
                        ps[:, 0:HF],
                        x_ap(xa, j, 128 * s, 128 * (s + 1)),
                        wv_ap("wv" + wp, j),
                        start=(pi == 0 and j == 0),
                        stop=(pi == 2 and j == KD // 2 - 1),
                        perf_mode=DR)
            nc.vector.tensor_copy(
                v_sb[s][:].rearrange(
                    "p (h c) -> p h c", c=DK + 1)[:, :, 0:DK],
                ps[:, 0:HF].rearrange("p (h c) -> p h c", c=DK))

        def rope_one(fe, fo, dst, pool, eng, nch):
            sl = bass.ts(nch, 512)
            ct, st_ = cos_t(nch), sin_t(nch)
            t1 = pool.tile([128, 512], BF16, tag="rt", name="t1")
            eng.tensor_mul(t1[:], fe[:, sl], ct)
            t2 = pool.tile([128, 512], BF16, tag="rt", name="t2")
            eng.tensor_mul(t2[:], fo[:, sl], st_)
            eng.tensor_sub(dst[:, sl], t1[:], t2[:])
            t3 = pool.tile([128, 512], BF16, tag="rt", name="t3")
            eng.tensor_mul(t3[:], fe[:, sl], st_)
            t4 = pool.tile([128, 512], BF16, tag="rt", name="t4")
            eng.tensor_mul(t4[:], fo[:, sl], ct)
            eng.tensor_add(dst[:, S + 512 * nch:S + 512 * (nch + 1)],
                           t3[:], t4[:])

        def rope_q(nch):
            rope_one(qf[0], qf[1], qcd, rp, nc.vector, nch)

        def rope_k(nch):
            # k-side of late chunks on GpSimd (idle but ~3x slower per
            # op); early ones on DVE so scores aren't gated behind Pool
            if nch >= 1:
                rope_one(kf[0], kf[1], kcd, rpg, nc.gpsimd, nch)
            else:
                rope_one(kf[0], kf[1], kcd, rp, nc.vector, nch)

        def rope(nch):
            rope_k(nch)
            rope_q(nch)

        # oq column base per q-subblock: 0-1 in bank0, 2-3 in bank1;
        # denominators sit at col 64 of each 65-wide [O_h | d_h] strip
        OQC = (0, 256, 512, 768)

        def divide_qs(qr, hp, oq, q0, nq):
            # strided-AP reciprocal covers the 2*nq denominators at once
            rcn = dvp.tile([128, 2, 2], F32, tag="rc", name="rc", bufs=6)
            with nc.allow_low_precision("softmax denominator recip"):
                nc.vector.reciprocal(
                    rcn[:, 0:nq, :],
                    oq[:, 256 * q0:256 * (q0 + nq)].rearrange(
                        "p (a r) -> p a r", a=nq)[:, :, 64:130:65])
            for qs in range(nq):
                qsub = q0 + qs
                c0 = OQC[qsub]
                o_nat = dvp.tile([128, 128], BF16, tag="on", name="on",
                                 bufs=6)
                for h2 in range(2):
                    nc.vector.tensor_scalar_mul(
                        o_nat[:, 64 * h2:64 * h2 + 64],
                        oq[:, c0 + 65 * h2:c0 + 65 * h2 + 64],
                        rcn[:, qs, h2:h2 + 1])
                ptr = fp_tile_bf("ptr")
                nc.tensor.transpose(ptr[:, 0:128], o_nat[:], ident[:])
                s = 4 * qr + qsub
                nc.vector.tensor_copy(
                    otb[hp][:, 128 * s:128 * (s + 1)], ptr[:, 0:128])

        def outproj_s(s_):
            # one s-block: 2 accumulation chains, 2 evacuations, 1 DMA
            ob = osb.tile([128, 1024], BF16, tag="ob", name="ob", bufs=4)
            for nch in range(2):
                ps = fp_tile("op")
                for i2 in range(2):
                    nc.tensor.matmul(
                        ps[:], otb[i2][:, 128 * s_:128 * (s_ + 1)],
                        woa[:, 1024 * i2 + 512 * nch:
                             1024 * i2 + 512 * (nch + 1)],
                        start=(i2 == 0), stop=(i2 == 1))
                nc.vector.tensor_copy(
                    ob[:, 512 * nch:512 * (nch + 1)], ps[:])
            nc.sync.dma_start(out[128 * s_:128 * (s_ + 1), :], ob[:])

        # PE p-state warm-up: the ramp to full clock needs ~3us of
        # continuous busy; these dummy matmuls run inside the otherwise
        # idle wait for the first x transfer, so stage A starts warm
        warm = fp_tile("warm")
        for _ in range(34):
            nc.tensor.matmul(warm[0:64, 0:64], wall[:, 0:64],
                             wall[:, 64:128], start=True, stop=True)

        # ---- stage A: Q first (copies on the pre-Exp-idle ScalarE,
        # rope_q immediately), then K, so the first scores' dependency
        # chain is Q-products -> rope_q || K-products -> rope_k
        qaccs = [sc_tile("qa0"), sc_tile("qa1")]
        for acc, wn in ((qaccs[0], "wq"), (qaccs[1], "wk")):
            for pi, (xp, wp) in enumerate(PRODS):
                xa = xah if xp == "h" else xal
                for j in range(KD // 2):
                    for m in range(2):
                        nc.tensor.matmul(
                            acc[:, 512 * m:512 * (m + 1)],
                            w_ap(wn + wp, j, m), x_ap(xa, j, 0, 512),
                            start=(pi == 0 and j == 0),
                            stop=(pi == 2 and j == KD // 2 - 1),
                            perf_mode=DR)
            dst = qf if wn == "wq" else kf
            for m in range(2):
                nc.scalar.copy(dst[m][:, 0:512],
                               acc[:, 512 * m:512 * (m + 1)])
            if wn == "wq":
                rope_q(0)
            else:
                rope_k(0)
        v_group(0)

        # remaining projection work in <=900ns chains with rough PE-ns
        # costs, metered one-at-a-time between score emissions.
        # plist gates scores (rope deps), vlist gates AV (v_sb deps).
        plist = []
        for nch in range(1, NS):
            for m in range(2):
                plist.append((nch, 1290, lambda m=m, n=nch:
                              qk_group(qf, "wq", m, n)))
            plist.append((nch, 0, lambda n=nch: rope_q(n)))
            for m in range(2):
                plist.append((nch, 1290, lambda m=m, n=nch:
                              qk_group(kf, "wk", m, n)))
            plist.append((nch, 0, lambda n=nch: rope_k(n)))
        vlist = [(s // 4, 660, lambda s=s: v_group(s))
                 for s in range(1, SB)]

        deficit = [0.0]

        def flush_plist(gate):
            while plist and plist[0][0] <= gate:
                plist.pop(0)[2]()

        def flush_vlist(gate_s):
            while vlist and vlist[0][0] * 4 <= gate_s:
                vlist.pop(0)[2]()

        def meter(ns_budget, pgate, vgate):
            deficit[0] = min(deficit[0] + ns_budget, 4200.0)
            while True:
                if plist and plist[0][0] <= pgate and \
                        deficit[0] >= plist[0][1]:
                    lst = plist
                elif vlist and vlist[0][0] <= vgate and \
                        deficit[0] >= vlist[0][1]:
                    lst = vlist
                else:
                    return
                n, c, f = lst.pop(0)
                f()
                deficit[0] -= c

        # ---- attention + out-projection, software-pipelined ------------
        def scores_exp(qr, hp, kb):
            # per head: ONE fp8 DoubleRow matmul, contraction pairs
            # (even_i, odd_i) of 32 partitions x 2 free-blocks
            q0 = 512 * qr
            off = max(0, 128 * kb - q0)
            diag = kb >= 4 * qr
            sc2 = sc_tile("sc")
            for h2 in range(2):
                g = 2 * hp + h2
                hh = 512 * h2
                st = kcd[32 * g:32 * (g + 1), :].rearrange(
                    "p (t n) -> p t n", t=2)[:, :, 128 * kb:128 * (kb + 1)]
                mv = qcd[32 * g:32 * (g + 1), :].rearrange(
                    "p (t n) -> p t n", t=2)[:, :, q0 + off:q0 + 512]
                nc.tensor.matmul(
                    sc2[:, hh + off:hh + 512], st, mv,
                    start=True, stop=not diag, perf_mode=DR,
                    tile_position=(32 * g, 0),
                    skip_group_check=True)
                if diag:
                    # causal mask: accumulate -3.67e6 onto the strict
                    # upper triangle of the diagonal 128-block
                    nc.tensor.matmul(
                        sc2[:, hh + off:hh + off + 128],
                        ident_dr, trineg_dr,
                        start=False, stop=True, perf_mode=DR,
                        tile_position=(0, 0),
                        skip_group_check=True)
            at2 = atp.tile([128, 1024], BF16, tag="at", name="at2",
                           bufs=44)
            nc.scalar.activation(
                at2[:].rearrange("p (t c) -> p t c", t=2)[:, :, off:512],
                sc2[:].rearrange("p (t c) -> p t c", t=2)[:, :, off:512],
                AF.Exp, scale=EXPS)
            return at2, off

        def av(qr, hp, kb, at2, off):
            heads = (2 * hp, 2 * hp + 1)
            oq = oq_box[0]
            for qsub in range(max(0, kb - 4 * qr), 4):
                for h in heads:
                    h2 = h % 2
                    nc.tensor.matmul(
                        oq[:, OQC[qsub] + 65 * h2:
                           OQC[qsub] + 65 * h2 + 65],
                        at2[:, 512 * h2 + 128 * qsub:
                            512 * h2 + 128 * (qsub + 1)],
                        v_sb[kb][:, 65 * h:65 * h + 65],
                        start=(kb == 0 and h2 == 0 and qsub in (0, 2)),
                        stop=(kb == 4 * qr + qsub))
            if kb == 4 * qr + 1:
                divide_qs(qr, hp, oq, 0, 2)
                if hp == 1:
                    outproj_s(4 * qr)
                    outproj_s(4 * qr + 1)
            elif kb == 4 * qr + 2:
                divide_qs(qr, hp, oq, 2, 1)
                if hp == 1:
                    outproj_s(4 * qr + 2)

        # process q-ranges 0,1,3,2: the tail is pinned to the LAST
        # pair's late exps, so end on a shorter pair than qr3
        jobs = [(qr, hp, kb) for qr in (0, 1, 3, 2) for hp in range(2)
                for kb in range(4 * (qr + 1))]
        NJ = len(jobs)
        at2q = {}
        emitted = [0]
        tail = [None]
        oq_box = [None]

        def emit_next(avqr):
            j = emitted[0]
            qr_, hp_, kb_ = jobs[j]
            flush_plist(qr_)
            at2q[j] = scores_exp(qr_, hp_, kb_)
            emitted[0] += 1
            # exp engine-time of this job, minus the job's own PE cost,
            # is the metering budget for projection fillers; the gate
            # follows the AV side so pops can't outrun the x-DMA stream
            off_ = max(0, 128 * kb_ - 512 * qr_)
            meter(0.833 * 2 * (512 - off_) + 180 - 350,
                  qr_ + 1, qr_ + 1)

        while emitted[0] < min(PF, NJ):
            emit_next(0)
            meter(260.0, 1, 0)

        cur_pair = [None]
        for j in range(NJ):
            qr_, hp_, kb_ = jobs[j]
            if (qr_, hp_) != cur_pair[0]:
                if tail[0] is not None:
                    pqr, php, poq = tail[0]
                    divide_qs(pqr, php, poq, 3, 1)
                    if php == 1:
                        outproj_s(4 * pqr + 3)
                    tail[0] = None
                if qr_ == NS - 1:
                    oq_box[0] = psum.tile([128, 1024], F32, tag="sc",
                                          name="oqz", bufs=2)
                else:
                    oq_box[0] = oq_tile("oq")
                cur_pair[0] = (qr_, hp_)
            flush_vlist(kb_)
            if emitted[0] < NJ:
                emit_next(qr_)
            # drain the emission queue early (2 scores/step once every
            # x transfer has landed) so the final pairs' AV overlaps the
            # exp stream instead of trailing the last score by ~13us
            if j >= 1 and emitted[0] < NJ:
                emit_next(qr_)
            if j >= 2 and emitted[0] < NJ:
                emit_next(qr_)
            if j >= 3 and emitted[0] < NJ:
                emit_next(qr_)
            av(qr_, hp_, kb_, *at2q.pop(j))
            if kb_ == 4 * (qr_ + 1) - 1:
                tail[0] = (qr_, hp_, oq_box[0])

        pqr, php, poq = tail[0]
        divide_qs(pqr, php, poq, 3, 1)
        outproj_s(4 * pqr + 3)

    if not nc.is_finalized():
        nc.finalize()
    return nc


_CACHE = {}


def _get_nc():
    if "nc" not in _CACHE:
        _CACHE["nc"] = _build_program()
    return _CACHE["nc"]


# ---------------------------------------------------------------------------
# Host side: shard, run, gather
# ---------------------------------------------------------------------------
def _pack_w(w):
    # [1024, 256] -> SBUF-wide [128, 8*256] (k-chunks side by side)
    return np.ascontiguousarray(
        w.reshape(KD, 128, HF).transpose(1, 0, 2).reshape(128, KD * HF))


def _core_inputs(c, x, Wq, Wk, Wv, Wo, csA, c8A):
    b, hg = c // 4, c % 4
    bf = ml_dtypes.bfloat16
    xTc = np.ascontiguousarray(x[b].T).astype(bf)
    # feature permutation: [evens of h0..h3 | odds of h0..h3], head-major
    rows = []
    for par in (0, 1):
        for j in range(HPC):
            base = DK * (HPC * hg + j)
            rows += [base + 2 * i + par for i in range(DK // 2)]
    rows = np.asarray(rows)
    vcols = np.arange(HF) + HF * hg
    woTc = np.ascontiguousarray(Wo[:, vcols].T / 64.0)  # [256, 1024]
    woP = np.ascontiguousarray(
        woTc.reshape(2, 128, D).transpose(1, 0, 2).reshape(128, 2 * D))
    F8 = ml_dtypes.float8_e4m3
    xT32 = np.ascontiguousarray(x[b].T)
    xh = xT32.astype(F8)
    xl = (xT32 - xh.astype(np.float32)).astype(F8)
    wp = {}
    for wn, W, rr in (("wq", Wq, rows), ("wk", Wk, rows),
                      ("wv", Wv, vcols)):
        W64 = np.ascontiguousarray(64.0 * W[rr, :].T.astype(np.float32))
        Wh = W64.astype(F8)
        Wl = (W64 - Wh.astype(np.float32)).astype(F8)
        wp[wn + "h"] = _pack_w(Wh)
        wp[wn + "l"] = _pack_w(Wl)
    w8 = np.concatenate([wp["wqh"], wp["wkh"], wp["wql"], wp["wkl"],
                         wp["wvh"], wp["wvl"]], axis=1)
    return {"x8A": np.ascontiguousarray(np.concatenate([xh, xl], axis=0)),
            "w8A": np.ascontiguousarray(w8),
            "woA": woP.astype(bf), "csA": csA, "c8A": c8A}


def _run(x, Wq, Wk, Wv, Wo, token_positions, **spmd_kwargs):
    x = np.asarray(x, np.float32)
    Wq = np.asarray(Wq, np.float32)
    Wk = np.asarray(Wk, np.float32)
    Wv = np.asarray(Wv, np.float32)
    Wo = np.asarray(Wo, np.float32)
    pos = np.asarray(token_positions).astype(np.float32)

    inv = THETA ** (-np.arange(0, DK, 2, dtype=np.float32) / DK)  # [32]
    ang = pos[:, None] * inv[None, :]                             # [S, 32]
    cosT = np.tile((QKS / 64.0) * np.cos(ang).T, (4, 1))          # [128, S]
    sinT = np.tile((QKS / 64.0) * np.sin(ang).T, (4, 1))
    pieces = [cosT[:, 0:512], sinT[:, 0:512], np.eye(128, dtype=np.float32)]
    for n in range(1, 4):
        pieces += [cosT[:, 512 * n:512 * (n + 1)],
                   sinT[:, 512 * n:512 * (n + 1)]]
    csA = np.ascontiguousarray(
        np.concatenate(pieces, axis=1).astype(ml_dtypes.bfloat16))

    # fp8 DoubleRow mask consts: ident_dr[p,t,m] = 64*(p+64t==m) e4m3;
    # trineg_dr[p,t,n] = -57344*(p+64t > n) e5m2 (bytes in an e4m3 tensor)
    p = np.arange(64)[:, None, None]
    t = np.arange(2)[None, :, None]
    n = np.arange(128)[None, None, :]
    ident_dr = (64.0 * ((p + 64 * t) == n)).astype(np.float32)
    trineg = (-57344.0 * ((p + 64 * t) > n)).astype(np.float32)
    tri_e5 = trineg.astype(ml_dtypes.float8_e5m2)
    tri_as_e4 = tri_e5.view(np.uint8).view(ml_dtypes.float8_e4m3)
    c8A = np.zeros((128, 512), ml_dtypes.float8_e4m3)
    c8A[0:64, 0:256] = ident_dr.reshape(64, 256).astype(
        ml_dtypes.float8_e4m3)
    c8A[0:64, 256:512] = tri_as_e4.reshape(64, 256)
    c8A = np.ascontiguousarray(c8A)

    in_maps = [_core_inputs(c, x, Wq, Wk, Wv, Wo, csA, c8A)
               for c in range(N_CORES)]
    res = run_bass_kernel_spmd(_get_nc(), in_maps,
                               core_ids=list(range(N_CORES)), **spmd_kwargs)
    outf = np.zeros((B, S, D), np.float32)
    for c in range(N_CORES):
        outf[c // 4] += np.asarray(res.results[c]["out"], np.float32)
    return outf, res


def kernel(x, Wq, Wk, Wv, Wo, token_positions):
    outf, _ = _run(x, Wq, Wk, Wv, Wo, token_positions)
    return outf


# revision 8
# speedup vs baseline: 1.0509x; 1.0509x over previous
"""Causal multi-head attention + RoPE — Trainium2 Bass kernel, 8-core SPMD.

Sharding: batch (2) x head-groups (4 heads each) -> 8 cores.
Wq/Wk/Wv are column-sharded per head group, Wo row-sharded; each core
computes a partial out-projection [S, D] and the host sums the 4
partials per batch (the "all-reduce after out_proj").

Per-core device pipeline (projections bf16, scores fp8 DoubleRow):
  1. x^T lands via combined strided DMAs, first-512-seq-columns first,
     so Q+K projections for the first q-range run progressively off a
     ~3us transfer and the first Exp issues at ~8us. Weight rows are
     host-permuted head-major so even/odd feature pairs share a
     partition (qf0[32h+i] = q_{h,2i}, qf1[32h+i] = q_{h,2i+1}).
  2. RoPE (q-side DVE, k-side GpSimd) writes 16-scaled fp8 rotated
     halves straight into qcd/kcd [128, 2S] (re block | ro block).
     Scores = ONE fp8 DoubleRow matmul per head per k-block: the
     contraction pairs (even_i, odd_i) sit on one partition with the
     pair split along the free axis, so no head-regroup matmuls exist.
     Causal mask = a second fp8 DoubleRow matmul accumulating -3.67e6
     onto the diagonal block's upper triangle (Exp underflows to 0).
  3. ScalarE runs ONLY Exp (scores bounded => max-pass free), one
     merged instruction per k-block covering both heads. Exp'd tiles
     flow through a GLOBAL 16-deep cross-q-range prefetch queue (exp
     work is backloaded: the last q-range holds 40% of it). Remaining
     projection work is chopped into <=900ns chains metered one at a
     time between score emissions — the 2-buf score-tile rotation
     gives ScalarE only ~1.9us of runway, so any larger PE bunch
     would starve it.
  4. AV transposed in bf16: stationary = exp'd scores, moving =
     [V_h | 1]: O[q,f]+denominator accumulate in a 2-bank tile; one
     strided-AP reciprocal per PSUM bank (4 denominators at once),
     per-partition scalar-multiply normalizes, a PE transpose restores
     O^T feature-major. Bank-0 divides start two k-blocks before the
     pair ends; bank-1 + out-projection ride into the next pair.
  5. partial = O_norm @ Wo^T per 2-s-block slab with per-s-block bf16
     DMAs out; the host sums the 4 partials per batch.
"""

import os
import sys
from contextlib import ExitStack

import numpy as np

for _p in ("/opt/trn_rl_repo", "/root/.axon_site/_ro/trn_rl_repo"):
    if os.path.isdir(_p) and _p not in sys.path:
        sys.path.insert(0, _p)

import ml_dtypes  # noqa: E402
import concourse.bass as bass  # noqa: E402
import concourse.tile as tile  # noqa: E402
from concourse import bacc, mybir  # noqa: E402
from concourse.bass_utils import run_bass_kernel_spmd  # noqa: E402

BF16 = mybir.dt.bfloat16
F32 = mybir.dt.float32
F8E4 = mybir.dt.float8e4
F8E5 = mybir.dt.float8e5
AF = mybir.ActivationFunctionType
DR = mybir.MatmulPerfMode.DoubleRow

B, S, D = 2, 2048, 1024
H, DK = 16, 64
HPC = 4                # heads per core
HF = HPC * DK          # 256 projected features per core
N_CORES = 8
THETA = 10000.0
SCALE = 1.0 / (DK ** 0.5)
QKS = 16.0             # fp8 q/k prescale (folded into cos/sin tables)
EXPS = SCALE / (QKS * QKS)

KD = D // 128          # 8 contraction chunks for projections
NS = S // 512          # 4 q-ranges of 512
SB = S // 128          # 16 s-blocks of 128
PF = 22                # global exp'd-score prefetch lookahead (SBUF tiles)


# ---------------------------------------------------------------------------
# Device program (identical on all 8 cores; only the input shards differ)
# ---------------------------------------------------------------------------
def _build_program():
    nc = bacc.Bacc("TRN2", target_bir_lowering=False, debug=False,
                   num_devices=N_CORES)

    x8A = nc.dram_tensor("x8A", [2 * D, S], F8E4, kind="ExternalInput")
    w8A = nc.dram_tensor("w8A", [128, 6 * KD * HF], F8E4,
                         kind="ExternalInput")
    woA = nc.dram_tensor("woA", [128, 2 * D], BF16, kind="ExternalInput")
    # [cos0|sin0|ident | cos1|sin1 | cos2|sin2 | cos3|sin3] (x0.25)
    csA = nc.dram_tensor("csA", [128, 2 * S + 128], BF16,
                         kind="ExternalInput")
    # [ident_dr(2x128) | trineg_dr(2x128, e5m2 bytes)]
    c8A = nc.dram_tensor("c8A", [128, 512], F8E4, kind="ExternalInput")
    out = nc.dram_tensor("out", [S, D], BF16, kind="ExternalOutput")

    with tile.TileContext(nc) as tc, ExitStack() as ctx:
        cons = ctx.enter_context(tc.tile_pool(name="cons", bufs=1))

        # ---- persistent SBUF tensors -----------------------------------
        # x^T (fp8 value + fp8 residual) as chunk-major tiles so strided
        # DMAs can land the first 512 seq-cols of ALL 8 d-chunks at once
        xa8 = cons.tile([128, 2 * KD * S], F8E4, tag="xa8", name="xa8")
        xah, xal = xa8[:, 0:KD * S], xa8[:, KD * S:2 * KD * S]
        wall = cons.tile([128, 6 * KD * HF], F8E4, tag="wall", name="wall")
        WOFF = {"wqh": 0, "wkh": 2048, "wql": 4096, "wkl": 6144,
                "wvh": 8192, "wvl": 10240}
        wsb = {k: wall[:, WOFF[k]:WOFF[k] + KD * HF] for k in WOFF}
        woa = cons.tile([128, 2 * D], BF16, tag="woa", name="woa")
        csa = cons.tile([128, 2 * S + 128], BF16, tag="csa", name="csa")

        def cs_base(nch):
            return 1152 + 1024 * (nch - 1) if nch else 0

        def cos_t(nch):
            b = cs_base(nch)
            return csa[:, b:b + 512]

        def sin_t(nch):
            b = cs_base(nch) + 512
            return csa[:, b:b + 512]

        ident = csa[:, 1024:1152]
        c8a = cons.tile([128, 512], F8E4, tag="c8a", name="c8a")
        ident_dr = c8a[0:64, 0:256].rearrange("p (t n) -> p t n", t=2)
        trineg_dr = c8a.bitcast(F8E5)[0:64, 256:512].rearrange(
            "p (t n) -> p t n", t=2)
        # bf16 pre-RoPE staging (even/odd halves, head-major partitions)
        qf = [cons.tile([128, S], BF16, tag=f"qf{m}", name=f"qf{m}")
              for m in range(2)]
        kf = [cons.tile([128, S], BF16, tag=f"kf{m}", name=f"kf{m}")
              for m in range(2)]
        # fp8 rotated q/k: [re block | ro block], 16x prescaled
        qcd = cons.tile([128, 2 * S], F8E4, tag="qcd", name="qcd")
        kcd = cons.tile([128, 2 * S], F8E4, tag="kcd", name="kcd")
        # V in natural layout, 65 columns per head (denom col appended)
        v_sb = [cons.tile([128, HPC * (DK + 1)], BF16, tag=f"v{s}",
                          name=f"v{s}")
                for s in range(SB)]
        # normalized O^T (features x S), two 128-feature tiles
        otb = [cons.tile([128, S], BF16, tag=f"otb{i}", name=f"otb{i}")
               for i in range(2)]

        # ---- input DMAs ------------------------------------------------
        # consolidated + strictly need-ordered; x value/residual chunks
        # land per seq-range via one 3D-strided transfer each
        def x_dma_p(xpart, c0, c1):
            # one 3D-strided transfer for the 8 d-chunks of one seq-range
            # of the fp8 value (xpart=0) or residual (xpart=1) plane
            base = KD * S * xpart
            d = xa8[:, base:base + KD * S].rearrange(
                "p (k s) -> p k s", k=KD)[:, :, c0:c1]
            sr = x8A[D * xpart:D * (xpart + 1), c0:c1].rearrange(
                "(k p) s -> p k s", p=128)
            nc.sync.dma_start(d, sr)

        def x_dma(c0, c1):
            x_dma_p(0, c0, c1)
            x_dma_p(1, c0, c1)

        # first x range: value plane on SP, residual on DVE (parallel)

        w2 = wall[:, 0:4096].rearrange("p (u c) -> p u c", u=2)
        wA2 = w8A[:, 0:4096].rearrange("p (u c) -> p u c", u=2)
        # first d-chunk of wq8h+wk8h, then their rests, then wq8l+wk8l
        # the SP sequencer can only issue one DMA per ~650ns, so the
        # startup-critical transfers fan out across three issue queues
        # (ScalarE idles until the first Exp, DVE until the first copy)
        nc.scalar.dma_start(w2[:, :, 0:256], wA2[:, :, 0:256])
        x_dma_p(0, 0, 512)
        d0 = xa8[:, KD * S:2 * KD * S].rearrange(
            "p (k s) -> p k s", k=KD)[:, :, 0:512]
        s0 = x8A[D:2 * D, 0:512].rearrange("(k p) s -> p k s", p=128)
        nc.scalar.dma_start(d0, s0)                                       # SP + DVE
        nc.scalar.dma_start(w2[:, :, 256:2048], wA2[:, :, 256:2048])
        nc.sync.dma_start(wall[:, 4096:8192], w8A[:, 4096:8192])
        nc.scalar.dma_start(csa[:, 0:1152], csA[:, 0:1152])  # cs0+ident
        nc.scalar.dma_start(c8a[:], c8A[:, :])   # mask before first diag
        nc.sync.dma_start(wall[:, 8192:12288], w8A[:, 8192:12288])  # wv
        x_dma(512, 1024)
        nc.sync.dma_start(csa[:, 1152:2176], csA[:, 1152:2176])  # cs1
        nc.sync.dma_start(woa[:], woA[:, :])   # out-proj qr0
        x_dma(1024, 1536)
        nc.sync.dma_start(csa[:, 2176:3200], csA[:, 2176:3200])  # cs2
        x_dma(1536, 2048)
        nc.sync.dma_start(csa[:, 3200:4224], csA[:, 3200:4224])  # cs3

        psum = ctx.enter_context(tc.tile_pool(name="psum", bufs=2,
                                              space="PSUM"))
        atp = ctx.enter_context(tc.tile_pool(name="atp", bufs=44))
        dvp = ctx.enter_context(tc.tile_pool(name="dvp", bufs=6))
        rp = ctx.enter_context(tc.tile_pool(name="rope", bufs=4))
        rpg = ctx.enter_context(tc.tile_pool(name="ropeg", bufs=4))
        osb = ctx.enter_context(tc.tile_pool(name="osb", bufs=2))

        # denominator columns: memset all up front (Pool idles during
        # the input DMA anyway; v_group then only writes the V columns)
        for s_ in range(SB):
            nc.gpsimd.memset(v_sb[s_][:], 1.0)

        def sc_tile(name):
            return psum.tile([128, 1024], F32, tag="sc", name=name, bufs=2)

        def fp_tile_bf(name):
            # bf16 view of an "fp"-class slot (transpose staging) — keeps
            # the divide-path transposes OUT of the score-tile rotation,
            # which the Exp stream is latency-coupled to
            return psum.tile([128, 1024], BF16, tag="fp", name=name, bufs=2)

        def oq_tile(name):
            return psum.tile([128, 1024], F32, tag="oq", name=name, bufs=1)

        def fp_tile(name):
            return psum.tile([128, 512], F32, tag="fp", name=name, bufs=2)

        # ---- building blocks -------------------------------------------
        # DoubleRow access patterns: contraction chunk-pairs (2j, 2j+1)
        # sit at free-axis stride S (x) / HF (weights)
        def w_ap(w, j, m):
            return wsb[w][:, 512 * j:512 * (j + 1)].rearrange(
                "p (t c) -> p t c", t=2)[:, :, 128 * m:128 * (m + 1)]

        def wv_ap(w, j):
            return wsb[w][:, 512 * j:512 * (j + 1)].rearrange(
                "p (t c) -> p t c", t=2)

        def x_ap(xa, j, c0, c1):
            return xa[:, 2 * S * j:2 * S * (j + 1)].rearrange(
                "p (t n) -> p t n", t=2)[:, :, c0:c1]

        # psum = x8h@W8h + x8h@W8l + x8l@W8h  (fp8 error feedback,
        # ~bf16 accuracy at half the PE rows per product)
        PRODS = (("h", "h"), ("h", "l"), ("l", "h"))

        def qk_group(dst, wn, m, nch):
            c0 = 512 * nch
            ps = fp_tile("qkg")
            for pi, (xp, wp) in enumerate(PRODS):
                xa = xah if xp == "h" else xal
                for j in range(KD // 2):
                    nc.tensor.matmul(
                        ps[:], w_ap(wn + wp, j, m),
                        x_ap(xa, j, c0, c0 + 512),
                        start=(pi == 0 and j == 0),
                        stop=(pi == 2 and j == KD // 2 - 1),
                        perf_mode=DR)
            nc.vector.tensor_copy(dst[m][:, c0:c0 + 512], ps[:])

        def v_group(s):
            ps = fp_tile("vg")
            for pi, (xp, wp) in enumerate(PRODS):
                xa = xah if xp == "h" else xal
                for j in range(KD // 2):
                    nc.tensor.matmul(
                        ps[:, 0:HF],
                        x_ap(xa, j, 128 * s, 128 * (s + 1)),
                        wv_ap("wv" + wp, j),
                        start=(pi == 0 and j == 0),
                        stop=(pi == 2 and j == KD // 2 - 1),
                        perf_mode=DR)
            nc.vector.tensor_copy(
                v_sb[s][:].rearrange(
                    "p (h c) -> p h c", c=DK + 1)[:, :, 0:DK],
                ps[:, 0:HF].rearrange("p (h c) -> p h c", c=DK))

        def rope_one(fe, fo, dst, pool, eng, nch):
            sl = bass.ts(nch, 512)
            ct, st_ = cos_t(nch), sin_t(nch)
            t1 = pool.tile([128, 512], BF16, tag="rt", name="t1")
            eng.tensor_mul(t1[:], fe[:, sl], ct)
            t2 = pool.tile([128, 512], BF16, tag="rt", name="t2")
            eng.tensor_mul(t2[:], fo[:, sl], st_)
            eng.tensor_sub(dst[:, sl], t1[:], t2[:])
            t3 = pool.tile([128, 512], BF16, tag="rt", name="t3")
            eng.tensor_mul(t3[:], fe[:, sl], st_)
            t4 = pool.tile([128, 512], BF16, tag="rt", name="t4")
            eng.tensor_mul(t4[:], fo[:, sl], ct)
            eng.tensor_add(dst[:, S + 512 * nch:S + 512 * (nch + 1)],
                           t3[:], t4[:])

        def rope_q(nch):
            rope_one(qf[0], qf[1], qcd, rp, nc.vector, nch)

        def rope_k(nch):
            # k-side of late chunks on GpSimd (idle but ~3x slower per
            # op); early ones on DVE so scores aren't gated behind Pool
            if nch >= 1:
                rope_one(kf[0], kf[1], kcd, rpg, nc.gpsimd, nch)
            else:
                rope_one(kf[0], kf[1], kcd, rp, nc.vector, nch)

        def rope(nch):
            rope_k(nch)
            rope_q(nch)

        # oq column base per q-subblock: 0-1 in bank0, 2-3 in bank1;
        # denominators sit at col 64 of each 65-wide [O_h | d_h] strip
        OQC = (0, 256, 512, 768)

        def divide_qs(qr, hp, oq, q0, nq):
            # strided-AP reciprocal covers the 2*nq denominators at once
            rcn = dvp.tile([128, 2, 2], F32, tag="rc", name="rc", bufs=6)
            with nc.allow_low_precision("softmax denominator recip"):
                nc.vector.reciprocal(
                    rcn[:, 0:nq, :],
                    oq[:, 256 * q0:256 * (q0 + nq)].rearrange(
                        "p (a r) -> p a r", a=nq)[:, :, 64:130:65])
            for qs in range(nq):
                qsub = q0 + qs
                c0 = OQC[qsub]
                o_nat = dvp.tile([128, 128], BF16, tag="on", name="on",
                                 bufs=6)
                for h2 in range(2):
                    nc.vector.tensor_scalar_mul(
                        o_nat[:, 64 * h2:64 * h2 + 64],
                        oq[:, c0 + 65 * h2:c0 + 65 * h2 + 64],
                        rcn[:, qs, h2:h2 + 1])
                ptr = fp_tile_bf("ptr")
                nc.tensor.transpose(ptr[:, 0:128], o_nat[:], ident[:])
                s = 4 * qr + qsub
                nc.vector.tensor_copy(
                    otb[hp][:, 128 * s:128 * (s + 1)], ptr[:, 0:128])

        def outproj_s(s_):
            # one s-block: 2 accumulation chains, 2 evacuations, 1 DMA
            ob = osb.tile([128, 1024], BF16, tag="ob", name="ob", bufs=4)
            for nch in range(2):
                ps = fp_tile("op")
                for i2 in range(2):
                    nc.tensor.matmul(
                        ps[:], otb[i2][:, 128 * s_:128 * (s_ + 1)],
                        woa[:, 1024 * i2 + 512 * nch:
                             1024 * i2 + 512 * (nch + 1)],
                        start=(i2 == 0), stop=(i2 == 1))
                nc.vector.tensor_copy(
                    ob[:, 512 * nch:512 * (nch + 1)], ps[:])
            nc.sync.dma_start(out[128 * s_:128 * (s_ + 1), :], ob[:])

        # PE p-state warm-up: the ramp to full clock needs ~3us of
        # continuous busy; these dummy matmuls run inside the otherwise
        # idle wait for the first x transfer, so stage A starts warm
        warm = fp_tile("warm")
        for _ in range(52):
            nc.tensor.matmul(warm[0:64, 0:64], wall[:, 0:64],
                             wall[:, 64:128], start=True, stop=True)

        # ---- stage A: Q first (copies on the pre-Exp-idle ScalarE,
        # rope_q immediately), then K, so the first scores' dependency
        # chain is Q-products -> rope_q || K-products -> rope_k
        qaccs = [sc_tile("qa0"), sc_tile("qa1")]
        for acc, wn in ((qaccs[0], "wq"), (qaccs[1], "wk")):
            for pi, (xp, wp) in enumerate(PRODS):
                xa = xah if xp == "h" else xal
                for j in range(KD // 2):
                    for m in range(2):
                        nc.tensor.matmul(
                            acc[:, 512 * m:512 * (m + 1)],
                            w_ap(wn + wp, j, m), x_ap(xa, j, 0, 512),
                            start=(pi == 0 and j == 0),
                            stop=(pi == 2 and j == KD // 2 - 1),
                            perf_mode=DR)
            dst = qf if wn == "wq" else kf
            for m in range(2):
                nc.scalar.copy(dst[m][:, 0:512],
                               acc[:, 512 * m:512 * (m + 1)])
            if wn == "wq":
                rope_q(0)
            else:
                rope_k(0)
        v_group(0)

        # remaining projection work in <=900ns chains with rough PE-ns
        # costs, metered one-at-a-time between score emissions.
        # plist gates scores (rope deps), vlist gates AV (v_sb deps).
        plist = []
        for nch in range(1, NS):
            for m in range(2):
                plist.append((nch, 1290, lambda m=m, n=nch:
                              qk_group(qf, "wq", m, n)))
            plist.append((nch, 0, lambda n=nch: rope_q(n)))
            for m in range(2):
                plist.append((nch, 1290, lambda m=m, n=nch:
                              qk_group(kf, "wk", m, n)))
            plist.append((nch, 0, lambda n=nch: rope_k(n)))
        vlist = [(s // 4, 660, lambda s=s: v_group(s))
                 for s in range(1, SB)]

        deficit = [0.0]

        def flush_plist(gate):
            while plist and plist[0][0] <= gate:
                plist.pop(0)[2]()

        def flush_vlist(gate_s):
            while vlist and vlist[0][0] * 4 <= gate_s:
                vlist.pop(0)[2]()

        def meter(ns_budget, pgate, vgate):
            deficit[0] = min(deficit[0] + ns_budget, 4200.0)
            while True:
                if plist and plist[0][0] <= pgate and \
                        deficit[0] >= plist[0][1]:
                    lst = plist
                elif vlist and vlist[0][0] <= vgate and \
                        deficit[0] >= vlist[0][1]:
                    lst = vlist
                else:
                    return
                n, c, f = lst.pop(0)
                f()
                deficit[0] -= c

        # ---- attention + out-projection, software-pipelined ------------
        def scores_exp(qr, hp, kb):
            # per head: ONE fp8 DoubleRow matmul, contraction pairs
            # (even_i, odd_i) of 32 partitions x 2 free-blocks
            q0 = 512 * qr
            off = max(0, 128 * kb - q0)
            diag = kb >= 4 * qr
            sc2 = sc_tile("sc")
            for h2 in range(2):
                g = 2 * hp + h2
                hh = 512 * h2
                st = kcd[32 * g:32 * (g + 1), :].rearrange(
                    "p (t n) -> p t n", t=2)[:, :, 128 * kb:128 * (kb + 1)]
                mv = qcd[32 * g:32 * (g + 1), :].rearrange(
                    "p (t n) -> p t n", t=2)[:, :, q0 + off:q0 + 512]
                nc.tensor.matmul(
                    sc2[:, hh + off:hh + 512], st, mv,
                    start=True, stop=not diag, perf_mode=DR,
                    tile_position=(32 * g, 0),
                    skip_group_check=True)
                if diag:
                    # causal mask: accumulate -3.67e6 onto the strict
                    # upper triangle of the diagonal 128-block
                    nc.tensor.matmul(
                        sc2[:, hh + off:hh + off + 128],
                        ident_dr, trineg_dr,
                        start=False, stop=True, perf_mode=DR,
                        tile_position=(0, 0),
                        skip_group_check=True)
            at2 = atp.tile([128, 1024], BF16, tag="at", name="at2",
                           bufs=44)
            nc.scalar.activation(
                at2[:].rearrange("p (t c) -> p t c", t=2)[:, :, off:512],
                sc2[:].rearrange("p (t c) -> p t c", t=2)[:, :, off:512],
                AF.Exp, scale=EXPS)
            return at2, off

        def av(qr, hp, kb, at2, off):
            heads = (2 * hp, 2 * hp + 1)
            oq = oq_box[0]
            for qsub in range(max(0, kb - 4 * qr), 4):
                for h in heads:
                    h2 = h % 2
                    nc.tensor.matmul(
                        oq[:, OQC[qsub] + 65 * h2:
                           OQC[qsub] + 65 * h2 + 65],
                        at2[:, 512 * h2 + 128 * qsub:
                            512 * h2 + 128 * (qsub + 1)],
                        v_sb[kb][:, 65 * h:65 * h + 65],
                        start=(kb == 0 and h2 == 0 and qsub in (0, 2)),
                        stop=(kb == 4 * qr + qsub))
            if kb == 4 * qr + 1:
                divide_qs(qr, hp, oq, 0, 2)
                if hp == 1:
                    outproj_s(4 * qr)
                    outproj_s(4 * qr + 1)
            elif kb == 4 * qr + 2:
                divide_qs(qr, hp, oq, 2, 1)
                if hp == 1:
                    outproj_s(4 * qr + 2)

        # process q-ranges 0,1,3,2: the tail is pinned to the LAST
        # pair's late exps, so end on a shorter pair than qr3
        jobs = [(qr, hp, kb) for qr in (0, 1, 3, 2) for hp in range(2)
                for kb in range(4 * (qr + 1))]
        NJ = len(jobs)
        at2q = {}
        emitted = [0]
        tail = [None]
        oq_box = [None]

        def emit_next(avqr):
            j = emitted[0]
            qr_, hp_, kb_ = jobs[j]
            flush_plist(qr_)
            at2q[j] = scores_exp(qr_, hp_, kb_)
            emitted[0] += 1
            # exp engine-time of this job, minus the job's own PE cost,
            # is the metering budget for projection fillers; the gate
            # follows the AV side so pops can't outrun the x-DMA stream
            off_ = max(0, 128 * kb_ - 512 * qr_)
            meter(0.833 * 2 * (512 - off_) + 180 - 350,
                  qr_ + 1, qr_ + 1)

        while emitted[0] < min(PF, NJ):
            emit_next(0)
            meter(260.0, 1, 0)

        cur_pair = [None]
        for j in range(NJ):
            qr_, hp_, kb_ = jobs[j]
            if (qr_, hp_) != cur_pair[0]:
                if tail[0] is not None:
                    pqr, php, poq = tail[0]
                    divide_qs(pqr, php, poq, 3, 1)
                    if php == 1:
                        outproj_s(4 * pqr + 3)
                    tail[0] = None
                if qr_ == NS - 1:
                    oq_box[0] = psum.tile([128, 1024], F32, tag="sc",
                                          name="oqz", bufs=2)
                else:
                    oq_box[0] = oq_tile("oq")
                cur_pair[0] = (qr_, hp_)
            flush_vlist(kb_)
            if emitted[0] < NJ:
                emit_next(qr_)
            # drain the emission queue early (2 scores/step once every
            # x transfer has landed) so the final pairs' AV overlaps the
            # exp stream instead of trailing the last score by ~13us
            if j >= 1 and emitted[0] < NJ:
                emit_next(qr_)
            if j >= 2 and emitted[0] < NJ:
                emit_next(qr_)
            if j >= 3 and emitted[0] < NJ:
                emit_next(qr_)
            av(qr_, hp_, kb_, *at2q.pop(j))
            if kb_ == 4 * (qr_ + 1) - 1:
                tail[0] = (qr_, hp_, oq_box[0])

        pqr, php, poq = tail[0]
        divide_qs(pqr, php, poq, 3, 1)
        outproj_s(4 * pqr + 3)

    if not nc.is_finalized():
        nc.finalize()
    return nc


_CACHE = {}


def _get_nc():
    if "nc" not in _CACHE:
        _CACHE["nc"] = _build_program()
    return _CACHE["nc"]


# ---------------------------------------------------------------------------
# Host side: shard, run, gather
# ---------------------------------------------------------------------------
def _pack_w(w):
    # [1024, 256] -> SBUF-wide [128, 8*256] (k-chunks side by side)
    return np.ascontiguousarray(
        w.reshape(KD, 128, HF).transpose(1, 0, 2).reshape(128, KD * HF))


def _core_inputs(c, x, Wq, Wk, Wv, Wo, csA, c8A):
    b, hg = c // 4, c % 4
    bf = ml_dtypes.bfloat16
    xTc = np.ascontiguousarray(x[b].T).astype(bf)
    # feature permutation: [evens of h0..h3 | odds of h0..h3], head-major
    rows = []
    for par in (0, 1):
        for j in range(HPC):
            base = DK * (HPC * hg + j)
            rows += [base + 2 * i + par for i in range(DK // 2)]
    rows = np.asarray(rows)
    vcols = np.arange(HF) + HF * hg
    woTc = np.ascontiguousarray(Wo[:, vcols].T / 64.0)  # [256, 1024]
    woP = np.ascontiguousarray(
        woTc.reshape(2, 128, D).transpose(1, 0, 2).reshape(128, 2 * D))
    F8 = ml_dtypes.float8_e4m3
    xT32 = np.ascontiguousarray(x[b].T)
    xh = xT32.astype(F8)
    xl = (xT32 - xh.astype(np.float32)).astype(F8)
    wp = {}
    for wn, W, rr in (("wq", Wq, rows), ("wk", Wk, rows),
                      ("wv", Wv, vcols)):
        W64 = np.ascontiguousarray(64.0 * W[rr, :].T.astype(np.float32))
        Wh = W64.astype(F8)
        Wl = (W64 - Wh.astype(np.float32)).astype(F8)
        wp[wn + "h"] = _pack_w(Wh)
        wp[wn + "l"] = _pack_w(Wl)
    w8 = np.concatenate([wp["wqh"], wp["wkh"], wp["wql"], wp["wkl"],
                         wp["wvh"], wp["wvl"]], axis=1)
    return {"x8A": np.ascontiguousarray(np.concatenate([xh, xl], axis=0)),
            "w8A": np.ascontiguousarray(w8),
            "woA": woP.astype(bf), "csA": csA, "c8A": c8A}


def _run(x, Wq, Wk, Wv, Wo, token_positions, **spmd_kwargs):
    x = np.asarray(x, np.float32)
    Wq = np.asarray(Wq, np.float32)
    Wk = np.asarray(Wk, np.float32)
    Wv = np.asarray(Wv, np.float32)
    Wo = np.asarray(Wo, np.float32)
    pos = np.asarray(token_positions).astype(np.float32)

    inv = THETA ** (-np.arange(0, DK, 2, dtype=np.float32) / DK)  # [32]
    ang = pos[:, None] * inv[None, :]                             # [S, 32]
    cosT = np.tile((QKS / 64.0) * np.cos(ang).T, (4, 1))          # [128, S]
    sinT = np.tile((QKS / 64.0) * np.sin(ang).T, (4, 1))
    pieces = [cosT[:, 0:512], sinT[:, 0:512], np.eye(128, dtype=np.float32)]
    for n in range(1, 4):
        pieces += [cosT[:, 512 * n:512 * (n + 1)],
                   sinT[:, 512 * n:512 * (n + 1)]]
    csA = np.ascontiguousarray(
        np.concatenate(pieces, axis=1).astype(ml_dtypes.bfloat16))

    # fp8 DoubleRow mask consts: ident_dr[p,t,m] = 64*(p+64t==m) e4m3;
    # trineg_dr[p,t,n] = -57344*(p+64t > n) e5m2 (bytes in an e4m3 tensor)
    p = np.arange(64)[:, None, None]
    t = np.arange(2)[None, :, None]
    n = np.arange(128)[None, None, :]
    ident_dr = (64.0 * ((p + 64 * t) == n)).astype(np.float32)
    trineg = (-57344.0 * ((p + 64 * t) > n)).astype(np.float32)
    tri_e5 = trineg.astype(ml_dtypes.float8_e5m2)
    tri_as_e4 = tri_e5.view(np.uint8).view(ml_dtypes.float8_e4m3)
    c8A = np.zeros((128, 512), ml_dtypes.float8_e4m3)
    c8A[0:64, 0:256] = ident_dr.reshape(64, 256).astype(
        ml_dtypes.float8_e4m3)
    c8A[0:64, 256:512] = tri_as_e4.reshape(64, 256)
    c8A = np.ascontiguousarray(c8A)

    in_maps = [_core_inputs(c, x, Wq, Wk, Wv, Wo, csA, c8A)
               for c in range(N_CORES)]
    res = run_bass_kernel_spmd(_get_nc(), in_maps,
                               core_ids=list(range(N_CORES)), **spmd_kwargs)
    outf = np.zeros((B, S, D), np.float32)
    for c in range(N_CORES):
        outf[c // 4] += np.asarray(res.results[c]["out"], np.float32)
    return outf, res


def kernel(x, Wq, Wk, Wv, Wo, token_positions):
    outf, _ = _run(x, Wq, Wk, Wv, Wo, token_positions)
    return outf
